# revision 32
# baseline (speedup 1.0000x reference)
"""Trainium2 Bass kernel for nn_MultiHeadAttention (B=4,T=2048,C=1024,H=16,D=64).

Sharding: tensor-parallel over heads. 8 cores x 2 heads each.
Per core: QKV column slices (128 dims), full attention for its 2 heads,
Wo row slice -> bf16 partial output summed on host.

v5 design (v4 baseline measured 667us; trace: ACT exp 285us but first exp
at t=122us and 163us of ACT gaps; PE 232us at half-clock from HAM
oscillation; DVE 345us):
- ACT is pure exp (all PSUM evacuations on DVE): exp stream = 73.4us/batch
  is the metronome; everything else hides around it.
- Prologue: 4 proj chunks on 4 separate PSUM tags (sgA/sgB/ot0/ot1 slots,
  free before attention) -> dense PE, no RoPE ping-pong; V-pass overlaps
  the rsqrt DRAM roundtrip; k-applies before q-applies.
- RoPE: rot-half = one full-width mul by host-PRE-PERMUTED sin, then 4
  cheap bf16 SBUF->SBUF 32-block shift copies (194ns, 4x mode) + GpSimd
  add. Was 4x 686ns partition-sliced muls from PSUM.
- ss-mm / V-transposes / apply-broadcasts moved from sgA/sgB (collided
  with live attention S tiles) to mm512.
- ot tiles are [128,512]-shaped (1 bank; PV writes rows 0:65): row
  slices then behave on DVE ([65,512]-shaped tiles mis-read, probed).
  ytb[64:128] <- ot1[0:64] direct DVE copy (+64 shift probed OK), no
  stg+DMA roundtrip.
- sumexp normalize split: ot evacuation inline at qc end; rcp + bmm
  broadcast (one [128,512] psum, h1 at base-partition 64) + ytb muls
  deferred as the FIRST filler of the next qc -> no qc-boundary stall.
- Attention group order: S(g+1) | PV(g,h0) | filler | PV(g,h1) | filler.
- p3: both oc psums evacuated into one [128,2,512] bf16 ob tile, ONE
  row-contiguous DMA per token tile; out partials bf16 (halves DMA).
- exp table preloaded via dummy activation at t=0.

Probed pitfalls (do not regress): DVE APs cannot have strided or
stride-0 partition dims; reciprocal_approx_* input must start at
partition 0; TRN2 matmul output must be f32; single-DMA 2-block
partition swaps transfer wrong elements (use 4 contiguous DMAs or DVE).
"""
import sys

sys.path.insert(0, "/opt/trn_rl_repo")
import numpy as np
import ml_dtypes

BF16NP = ml_dtypes.bfloat16

N_CORES = 8
B_FULL, T_FULL, C = 4, 2048, 1024
H, D = 16, 64
HPC = H // N_CORES          # heads per core = 2
M2 = HPC * D                # 128
EPS = 1e-6

_NC_CACHE: dict = {}


def build_nc(B: int, T: int):
    import concourse.bass as bass
    import concourse.mybir as mybir
    from concourse import bacc
    from concourse.tile import TileContext

    BF16 = mybir.dt.bfloat16
    F32 = mybir.dt.float32
    AF = mybir.ActivationFunctionType
    ALU = mybir.AluOpType

    TT = B * T
    NCIN = C // 128             # 8 contraction tiles for projections
    CPB = T // 512              # 4 chunks of 512 tokens per batch
    NKT = T // 128              # 16 key tiles per batch
    NQC = T // 512              # 4 q chunks per batch
    NG = NKT // 2               # 8 key groups (KGS=2) per q chunk

    nc = bacc.Bacc("TRN2", target_bir_lowering=False, debug=False,
                   num_devices=N_CORES)

    xT_d = nc.dram_tensor("xT", [128, C // 128, TT], BF16,
                          kind="ExternalInput")
    wq_d = nc.dram_tensor("wq", [128, NCIN, M2], BF16, kind="ExternalInput")
    wk_d = nc.dram_tensor("wk", [128, NCIN, M2], BF16, kind="ExternalInput")
    wv_d = nc.dram_tensor("wv", [128, NCIN, M2], BF16, kind="ExternalInput")
    wo_d = nc.dram_tensor("wo", [M2, C], BF16, kind="ExternalInput")
    cos_d = nc.dram_tensor("cos2", [M2, T], F32, kind="ExternalInput")
    sinp_d = nc.dram_tensor("sin2p", [M2, T], F32, kind="ExternalInput")
    ident_d = nc.dram_tensor("ident", [128, 128], BF16, kind="ExternalInput")
    ones4q_d = nc.dram_tensor("ones4q", [128, 4], BF16, kind="ExternalInput")
    ones4k_d = nc.dram_tensor("ones4k", [128, 4], BF16, kind="ExternalInput")
    sel2_d = nc.dram_tensor("sel2", [2, 128], BF16, kind="ExternalInput")
    ones66_d = nc.dram_tensor("ones66", [128, T // 128, 66], BF16,
                              kind="ExternalInput")
    out_d = nc.dram_tensor("out", [TT, C], BF16, kind="ExternalOutput")

    with TileContext(nc) as tc:
        with (
            tc.tile_pool(name="const", bufs=1) as cp,
            tc.tile_pool(name="big", bufs=2) as bigp,
            tc.tile_pool(name="xs", bufs=4) as xsp,
            tc.tile_pool(name="attn", bufs=2) as atp,
            tc.tile_pool(name="scr", bufs=2) as scp,
            tc.tile_pool(name="drs", bufs=2, space="DRAM") as drp,
            tc.tile_pool(name="ps", bufs=1, space="PSUM") as psp,
        ):
            # exp table preload: tiny dummy activation fires immediately
            warm_in = cp.tile([1, 16], F32, tag="warm_in")
            warm_out = cp.tile([1, 16], BF16, tag="warm_out")
            nc.vector.memset(warm_in, 0.0)
            nc.scalar.activation(warm_out, warm_in, AF.Exp, scale=1.0)

            wq_sb = cp.tile([128, NCIN, M2], BF16, tag="wq")
            wk_sb = cp.tile([128, NCIN, M2], BF16, tag="wk")
            wv_sb = cp.tile([128, NCIN, M2], BF16, tag="wv")
            wo_sb = cp.tile([128, C], BF16, tag="wo")
            cos_sb = cp.tile([128, T], F32, tag="cos")
            sinp_sb = cp.tile([128, T], F32, tag="sinp")
            ident = cp.tile([128, 128], BF16, tag="ident")
            ones4q = cp.tile([128, 4], BF16, tag="ones4q")
            ones4k = cp.tile([128, 4], BF16, tag="ones4k")
            sel2 = cp.tile([2, 128], BF16, tag="sel2")

            # wq/wk first: the first proj MMs need only wq + x chunk 0
            # (x prefetches are emitted right after the consts below)
            nc.sync.dma_start(out=wq_sb, in_=wq_d[:, :, :])
            nc.sync.dma_start(out=wk_sb, in_=wk_d[:, :, :])

            # rotate-half shift: out block <- t block (within-head swap)
            ROT_BLOCKS = (((0, 32), (32, 64)), ((32, 64), (0, 32)),
                          ((64, 96), (96, 128)), ((96, 128), (64, 96)))

            st: dict = {}
            xq: dict = {}

            def get_state(b):
                if b in st:
                    return st[b]
                qtb = bigp.tile([128, T], BF16, tag="qtb")
                ktb = bigp.tile([128, T], BF16, tag="ktb")
                ytb = bigp.tile([128, T], BF16, tag="ytb")
                vsb = bigp.tile([128, NKT, 130], BF16, tag="vsb")
                ssqk = scp.tile([4, T], F32, tag="ssqk", bufs=1)
                sc_q = scp.tile([2, T], BF16, tag="sc_q", bufs=1)
                sc_k = scp.tile([2, T], BF16, tag="sc_k", bufs=1)
                sums_t = drp.tile([4, T], F32, tag="sums")
                scales_t = drp.tile([4, T], BF16, tag="scales")
                if b < 2:
                    # ones columns persist in the physical buffer; later
                    # batches reuse them (V copies never touch cols 64/129)
                    nc.sync.dma_start(out=vsb[:, :, 64:130],
                                      in_=ones66_d[:, :, :])
                s = dict(qtb=qtb, ktb=ktb, ytb=ytb, vsb=vsb, ssqk=ssqk,
                         sc_q=sc_q, sc_k=sc_k, sums_t=sums_t,
                         scales_t=scales_t)
                st[b] = s
                return s

            def prefetch_x(b, c, eng=None):
                """Issue the x-chunk DMA ahead of its consuming quantum."""
                if (b, c) in xq or b >= B or c >= CPB:
                    return
                x_sb = xsp.tile([128, NCIN, 512], BF16, tag="x")
                (eng or nc.sync).dma_start(
                    out=x_sb,
                    in_=xT_d[:, :, b * T + c * 512: b * T + (c + 1) * 512])
                xq[(b, c)] = x_sb

            def rope_emit(s, name, ps, cc, act_sq=False):
                """RoPE for one projected 512-chunk (DVE+GpSimd only).
                Returns the squared tile for the deferred ss matmul.
                act_sq: square on the (idle) ACT engine -- prologue only,
                where the serial GpSimd add+sq chain gates the first exp."""
                dkey = "qtb" if name == "q" else "ktb"
                dd = s[dkey][:, cc]
                nc.vector.tensor_mul(out=dd, in0=ps, in1=cos_sb[:, cc])
                rot_t = scp.tile([128, 512], BF16, tag="rot_t")
                nc.vector.tensor_mul(out=rot_t, in0=ps, in1=sinp_sb[:, cc])
                rots = scp.tile([128, 512], BF16, tag="rots")
                for (d0, d1), (s0, s1) in ROT_BLOCKS:
                    nc.vector.tensor_copy(out=rots[d0:d1], in_=rot_t[s0:s1])
                nc.gpsimd.tensor_add(out=dd, in0=dd, in1=rots)
                # bufs=8: lives until the ss quantum; prologue round-robins
                # 4 chunks x {q,k}
                sq = scp.tile([128, 512], BF16, tag="sq", bufs=8)
                if act_sq:
                    nc.scalar.activation(sq, dd, AF.Square, scale=1.0)
                else:
                    nc.gpsimd.tensor_mul(out=sq, in0=dd, in1=dd)
                return sq

            def p1_qk_gen(b, c, qtag="mm512", ktag="mm512", tbufs=2,
                          mm_step=2, act_sq=False):
                """Q/K projections + RoPE + fused q+k sumsq for one chunk."""
                s = get_state(b)
                cc = slice(c * 512, (c + 1) * 512)
                prefetch_x(b, c)
                x_sb = xq[(b, c)]
                yield
                sqs = {}
                for name, w_sb, tag in (("q", wq_sb, qtag),
                                        ("k", wk_sb, ktag)):
                    ps = psp.tile([128, 512], F32, tag=tag, bufs=tbufs,
                                  name=f"acc_{name}{b}_{c}")
                    for ci in range(NCIN):
                        nc.tensor.matmul(ps, w_sb[:, ci], x_sb[:, ci],
                                         start=(ci == 0), stop=(ci == NCIN - 1),
                                         skip_group_check=True)
                        if ci % mm_step == mm_step - 1 and ci != NCIN - 1:
                            yield
                    sqs[name] = rope_emit(s, name, ps, cc, act_sq=act_sq)
                    yield
                # deferred sumsq: one psum tile, q rows 0:2 / k rows 2:4
                # via zero-padded selectors + psum accumulation. Deferring
                # to its own quantum keeps the PE queue head from stalling
                # on the GpSimd squares.
                ps_ss = psp.tile([4, 512], F32, tag="mm512", bufs=2,
                                 name=f"ss{b}_{c}")
                nc.tensor.matmul(ps_ss, ones4q, sqs["q"], start=True,
                                 stop=False, skip_group_check=True)
                nc.tensor.matmul(ps_ss, ones4k, sqs["k"], start=False,
                                 stop=True, skip_group_check=True)
                nc.vector.tensor_copy(out=s["ssqk"][:, cc], in_=ps_ss)
                if c == CPB - 1:
                    nc.sync.dma_start(out=s["sums_t"][:, :], in_=s["ssqk"])
                yield

            def p1_rsqrt_gen(b):
                """rsqrt of mean-square, DVE-only (magic constant + 2 Newton
                steps; no ACT table switch), then scales roundtrip."""
                import concourse.mybir as mybir
                I32 = mybir.dt.int32
                s = get_state(b)
                FP = 4 * T // 128
                pk = scp.tile([128, 5, FP], F32, tag="pk", bufs=1)
                y1b = scp.tile([128, FP], BF16, tag="y1b", bufs=1)
                nc.sync.dma_start(
                    out=pk[:, 0],
                    in_=s["sums_t"][:].rearrange("a t -> (a t)")
                    .rearrange("(p f) -> p f", p=128))
                ms, g, t1, tmp = (pk[:, j] for j in range(1, 5))
                nc.vector.tensor_scalar(out=ms, in0=pk[:, 0], scalar1=1.0 / D,
                                        scalar2=EPS, op0=ALU.mult,
                                        op1=ALU.add)
                # g0 via 0x5f3759df bit trick, then 2 Newton iterations
                nc.vector.tensor_scalar(
                    out=tmp.bitcast(I32), in0=ms.bitcast(I32), scalar1=1,
                    scalar2=0, op0=ALU.logical_shift_right,
                    op1=ALU.bitwise_or)
                nc.vector.tensor_scalar(
                    out=g.bitcast(I32), in0=tmp.bitcast(I32), scalar1=-1,
                    scalar2=0x5F3759DF, op0=ALU.mult, op1=ALU.add)
                for it in range(2):
                    nc.vector.tensor_mul(out=t1, in0=g, in1=g)
                    nc.vector.tensor_mul(out=t1, in0=t1, in1=ms)
                    nc.vector.tensor_scalar(out=t1, in0=t1, scalar1=-0.5,
                                            scalar2=1.5, op0=ALU.mult,
                                            op1=ALU.add)
                    out_ap = g if it == 0 else y1b
                    nc.vector.tensor_mul(out=out_ap, in0=g, in1=t1)
                nc.sync.dma_start(
                    out=s["scales_t"][:].rearrange("a t -> (a t)")
                    .rearrange("(p f) -> p f", p=128),
                    in_=y1b)
                nc.sync.dma_start(out=s["sc_q"][:, :], in_=s["scales_t"][0:2, :])
                nc.scalar.dma_start(out=s["sc_k"][:, :],
                                    in_=s["scales_t"][2:4, :])
                yield

            def p1_v_gen(b, c, vtag="mm512", tbufs=2, mm_step=2):
                """V projection + transpose into vsb for one chunk."""
                s = get_state(b)
                vsb = s["vsb"]
                x_sb = xq.pop((b, c))
                ps = psp.tile([128, 512], F32, tag=vtag, bufs=tbufs,
                              name=f"acc_v{b}_{c}")
                for ci in range(NCIN):
                    nc.tensor.matmul(ps, wv_sb[:, ci], x_sb[:, ci],
                                     start=(ci == 0), stop=(ci == NCIN - 1),
                                     skip_group_check=True)
                    if ci % mm_step == mm_step - 1 and ci != NCIN - 1:
                        yield
                # bufs=4: the prologue round-robins 4 chunks; with 2 slots
                # vtmp_c2 waits transposes_c0 whose mm512 release sits
                # behind vtmp_c2 in the strict DVE FIFO -> deadlock.
                vtmp = scp.tile([128, 512], BF16, tag="vtmp", bufs=4)
                nc.vector.tensor_copy(out=vtmp, in_=ps)
                yield
                for i in range(4):
                    kt_idx = c * 4 + i
                    ps_t = psp.tile([128, 128], BF16, tag="mm512", bufs=2,
                                    name=f"tp{b}_{kt_idx}")
                    nc.tensor.transpose(ps_t, vtmp[:, i * 128:(i + 1) * 128],
                                        ident)
                    # both head halves in one copy: cols {0:64, 65:129}
                    nc.vector.tensor_copy(
                        out=vsb[:, kt_idx, :]
                        .rearrange("p (h x) -> p h x", h=2)[:, :, 0:64],
                        in_=ps_t[:].rearrange("p (h i) -> p h i", h=2))
                    if i == 1:
                        yield

            def p1_apply_gen(b, order=("k", "q")):
                """Apply norm scales to qtb/ktb via outer-product bcasts.
                K first so attention's S matmuls unblock earlier."""
                s = get_state(b)
                for which in order:
                    dkey, sckey = (("qtb", "sc_q") if which == "q"
                                   else ("ktb", "sc_k"))
                    for c in range(CPB):
                        cc = slice(c * 512, (c + 1) * 512)
                        bps = psp.tile([128, 512], F32, tag="mm512", bufs=2,
                                       name=f"ap{b}{which}{c}")
                        nc.tensor.matmul(bps, sel2, s[sckey][:, cc],
                                         start=True, stop=True,
                                         skip_group_check=True)
                        nc.vector.tensor_mul(out=s[dkey][:, cc],
                                             in0=s[dkey][:, cc], in1=bps)
                        yield

            def p1_chain(b):
                return ([p1_qk_gen(b, c) for c in range(CPB)]
                        + [p1_rsqrt_gen(b)]
                        + [p1_v_gen(b, c) for c in range(CPB)]
                        + [p1_apply_gen(b)])

            def p3_chunk_gen(b, qc):
                """Wo projection + bf16 partial-output DMA, one DMA per tt."""
                s = get_state(b)
                for tt in range(qc * 4, qc * 4 + 4):
                    ob = scp.tile([128, 2, 512], BF16, tag="ob")
                    for oc in range(2):
                        pso = psp.tile([128, 512], F32, tag="mm512", bufs=2,
                                       name=f"wo{b}_{tt}_{oc}")
                        nc.tensor.matmul(
                            pso, s["ytb"][:, tt * 128:(tt + 1) * 128],
                            wo_sb[:, oc * 512:(oc + 1) * 512],
                            start=True, stop=True, skip_group_check=True)
                        nc.vector.tensor_copy(out=ob[:, oc], in_=pso)
                        if oc == 0:
                            yield
                    nc.sync.dma_start(
                        out=out_d[b * T + tt * 128: b * T + (tt + 1) * 128, :],
                        in_=ob.rearrange("p a f -> p (a f)"))
                    yield

            def norm_gen(b, qc, sml):
                """Deferred sumexp-normalize: rcp (bf16 staged) + one
                broadcast matmul (h1 at base partition 64) + 2 ytb muls."""
                s = get_state(b)
                qq = slice(qc * 512, (qc + 1) * 512)
                rcpf = scp.tile([1, 2, 512], F32, tag="rcpf")
                nc.vector.reciprocal_approx_fast(out=rcpf[:], in_=sml[:])
                rcpb = scp.tile([1, 2, 512], BF16, tag="rcpb")
                nc.vector.tensor_copy(out=rcpb, in_=rcpf)
                bps = psp.tile([128, 512], F32, tag="mm512", bufs=2,
                               name=f"nb{b}_{qc}")
                for h in range(2):
                    # sel2 row 0, cols 0:64 is an all-ones [1,64] selector;
                    # bf16 rhs keeps the MM at 1 cyc/row (f32 rhs is 4x)
                    nc.tensor.matmul(bps[h * 64:(h + 1) * 64],
                                     sel2[0:1, 0:64],
                                     rcpb[:, h], start=True, stop=True,
                                     skip_group_check=True)
                yield
                for h in range(2):
                    hs = slice(h * 64, (h + 1) * 64)
                    nc.vector.tensor_mul(out=s["ytb"][hs, qq],
                                         in0=s["ytb"][hs, qq],
                                         in1=bps[hs])
                yield

            def attn_qc(b, qc, fillers, prev_tail):
                """Attention for one 512-query chunk; KGS=2, heads on
                separate PSUM tags, pipelined by one key-group.

                The first group's S matmuls are emitted BEFORE the previous
                qc's ot-evacuation (prev_tail) so the exp stream never gaps
                at the qc boundary. Returns this qc's tail closure, which
                evacuates ot and queues [norm, p3] as a filler chain."""
                s = get_state(b)
                qtb, ktb, vsb, ytb = s["qtb"], s["ktb"], s["vsb"], s["ytb"]
                qq = slice(qc * 512, (qc + 1) * 512)
                # [128,512]-shaped tiles; PV writes rows 0:65 only
                ot = [psp.tile([128, 512], F32, tag=f"ot{h}", bufs=1,
                               name=f"ot{h}_{b}_{qc}")
                      for h in range(2)]

                def emit_sg_h(g, h):
                    hs = slice(h * 64, (h + 1) * 64)
                    sp = psp.tile([128, 2, 512], F32,
                                  tag=("sgA" if h == 0 else "sgB"),
                                  bufs=1, name=f"sg{h}")
                    for i in range(2):
                        ktg = g * 2 + i
                        nc.tensor.matmul(
                            sp[:, i],
                            ktb[hs, ktg * 128:(ktg + 1) * 128],
                            qtb[hs, qq],
                            start=True, stop=True, skip_group_check=True)
                    stexp = atp.tile([128, 2, 512], BF16,
                                     tag=("seA" if h == 0 else "seB"),
                                     name=f"se{h}")
                    nc.scalar.activation(stexp, sp, AF.Exp, scale=0.125)
                    return stexp

                def advance_filler():
                    while fillers:
                        chain = fillers[0]
                        try:
                            next(chain[0])
                            if len(fillers) > 1:
                                fillers.append(fillers.pop(0))
                            break
                        except StopIteration:
                            chain.pop(0)
                            if not chain:
                                fillers.pop(0)

                def pv(g, h, pend):
                    for i in range(2):
                        ktg = g * 2 + i
                        nc.tensor.matmul(
                            ot[h][0:65], vsb[:, ktg, h * 65:h * 65 + 65],
                            pend[h][:, i],
                            start=(ktg == 0), stop=(ktg == NKT - 1),
                            skip_group_check=True)

                # Pipelined by one group. Fillers go BEFORE each head's
                # S(g+1): both S(g+1,h) and PV(g,h) stall on exp(h,g)
                # completion (sgX slot release / stexp ready), so the
                # filler quantum absorbs that sem-latency window instead
                # of the PE queue head exposing it.
                pend = [emit_sg_h(0, 0), emit_sg_h(0, 1)]
                if prev_tail is not None:
                    fillers.insert(0, prev_tail())
                for g in range(NG):
                    nxt = [None, None]
                    advance_filler()
                    if g + 1 < NG:
                        nxt[0] = emit_sg_h(g + 1, 0)
                    pv(g, 0, pend)
                    advance_filler()
                    if g + 1 < NG:
                        nxt[1] = emit_sg_h(g + 1, 1)
                    pv(g, 1, pend)
                    advance_filler()
                    pend = nxt

                def tail():
                    # evacuate ot: sumexp rows + O rows (DVE only), then
                    # queue the deferred normalize + Wo chain as filler
                    sml = scp.tile([1, 2, 512], F32, tag="sml")
                    nc.vector.tensor_copy(out=sml[:, 0], in_=ot[0][64:65])
                    nc.vector.tensor_copy(out=sml[:, 1], in_=ot[1][64:65])
                    nc.vector.tensor_copy(out=ytb[0:64, qq], in_=ot[0][0:64])
                    nc.vector.tensor_copy(out=ytb[64:128, qq],
                                          in_=ot[1][0:64])
                    return [norm_gen(b, qc, sml), p3_chunk_gen(b, qc)]

                return tail

            # ---- prologue: batch 0 p1, dense PE via 4 separate psum tags
            PRO_TAGS = ("sgA", "sgB", "ot0", "ot1")

            def rr(gens):
                live = list(gens)
                while live:
                    for g in list(live):
                        try:
                            next(g)
                        except StopIteration:
                            live.remove(g)

            # cos/sin before x: the first RoPE needs them; x chunks spread
            # over both HWDGE queues (SP + ACT) to avoid serializing
            nc.sync.dma_start(out=cos_sb, in_=cos_d[:, :])
            nc.scalar.dma_start(out=sinp_sb, in_=sinp_d[:, :])
            for c in range(CPB):
                prefetch_x(0, c, eng=(nc.scalar if c % 2 else nc.sync))
            nc.scalar.dma_start(out=ones4q, in_=ones4q_d[:, :])
            nc.scalar.dma_start(out=ones4k, in_=ones4k_d[:, :])
            nc.sync.dma_start(out=wv_sb, in_=wv_d[:, :, :])
            nc.scalar.dma_start(out=ident, in_=ident_d[:, :])
            nc.scalar.dma_start(out=sel2, in_=sel2_d[:, :])
            nc.sync.dma_start(out=wo_sb, in_=wo_d[:, :])
            # pairs (not 4-wide): chunk 0's RoPE/squares chain starts ~2x
            # earlier, which gates the scales roundtrip -> first exp
            for pair in ((0, 1), (2, 3)):
                rr([p1_qk_gen(0, c, qtag=PRO_TAGS[c], ktag=PRO_TAGS[c],
                              tbufs=1, mm_step=4, act_sq=True)
                    for c in pair])
            rq = p1_rsqrt_gen(0)
            for _ in rq:
                pass
            for pair in ((0, 1), (2, 3)):
                rr([p1_v_gen(0, c, vtag=PRO_TAGS[c], tbufs=1, mm_step=4)
                    for c in pair])
            # apply LAST: its DVE muls wait on the scales DMA roundtrip;
            # emitting them earlier blocks the strict DVE FIFO (and its
            # mm512 bps allocs deadlock against the V transposes).
            for _ in p1_apply_gen(0):
                pass

            # ---- steady state ----
            tail = None
            for b in range(B):
                fillers = []
                if b + 1 < B:
                    fillers.append(p1_chain(b + 1))
                for qc in range(NQC):
                    if b + 1 < B and qc < CPB:
                        prefetch_x(b + 1, qc)
                    tail = attn_qc(b, qc, fillers, tail)
                # drain leftover filler quanta at the batch boundary
                for g in fillers:
                    for gen in g:
                        for _ in gen:
                            pass
                fillers.clear()
            # final qc's evacuation + norm + Wo
            if tail is not None:
                for gen in tail():
                    for _ in gen:
                        pass

    nc.compile()
    return nc


def make_core_inputs(x, cos, sin, Wq, Wk, Wv, Wo, B, T):
    """Host-side sharding. Returns list of 8 input dicts."""
    TT = B * T
    NCIN = C // 128
    # [128 ci, NCIN co, TT] so each x-chunk DMA reads contiguous rows
    xT = np.ascontiguousarray(
        np.asarray(x, np.float32).reshape(TT, C).T.astype(BF16NP)
        .reshape(NCIN, 128, TT).transpose(1, 0, 2))
    cosT = np.asarray(cos, np.float32).reshape(T, D).T      # [64, T]
    sinT = np.asarray(sin, np.float32).reshape(T, D).T
    sin_signed = np.concatenate([-sinT[0:32], sinT[32:64]], axis=0)
    # pre-permuted: sin2p[p] = sin_signed[partner(p)] (32-block swap)
    sin_perm = np.concatenate([sin_signed[32:64], sin_signed[0:32]], axis=0)
    cos2 = np.ascontiguousarray(np.concatenate([cosT, cosT], axis=0))
    sin2p = np.ascontiguousarray(np.concatenate([sin_perm, sin_perm],
                                                axis=0))
    ones4q = np.zeros((128, 4), BF16NP)
    ones4q[0:64, 0] = 1.0
    ones4q[64:128, 1] = 1.0
    ones4k = np.zeros((128, 4), BF16NP)
    ones4k[0:64, 2] = 1.0
    ones4k[64:128, 3] = 1.0
    sel2 = np.zeros((2, 128), BF16NP)
    sel2[0, 0:64] = 1.0
    sel2[1, 64:128] = 1.0

    def wprep(W, rows):
        # [128 ci, NCIN co, M2] so the weight DMA is contiguous
        wT = np.asarray(W, np.float32)[rows].T.astype(BF16NP)   # [C, M2]
        return np.ascontiguousarray(
            wT.reshape(NCIN, 128, M2).transpose(1, 0, 2))

    in_maps = []
    for core in range(N_CORES):
        rows = slice(core * M2, (core + 1) * M2)
        in_maps.append({
            "xT": xT,
            "wq": wprep(Wq, rows),
            "wk": wprep(Wk, rows),
            "wv": wprep(Wv, rows),
            "wo": np.ascontiguousarray(
                np.asarray(Wo, np.float32)[:, rows].T.astype(BF16NP)),
            "cos2": cos2,
            "sin2p": sin2p,
            "ident": np.eye(128, dtype=BF16NP),
            "ones4q": ones4q,
            "ones4k": ones4k,
            "sel2": sel2,
            "ones66": np.ones((128, T // 128, 66), BF16NP),
        })
    return in_maps


def kernel(x, cos, sin, Wq, Wk, Wv, Wo):
    from concourse.bass_utils import run_bass_kernel_spmd

    B, T = x.shape[0], x.shape[1]
    key = (B, T)
    if key not in _NC_CACHE:
        _NC_CACHE[key] = build_nc(B, T)
    nc = _NC_CACHE[key]
    in_maps = make_core_inputs(x, cos, sin, Wq, Wk, Wv, Wo, B, T)
    res = run_bass_kernel_spmd(nc, in_maps, core_ids=list(range(N_CORES)))
    out = np.zeros((B * T, C), np.float64)
    for r in res.results:
        out += r["out"].astype(np.float64)
    return out.astype(np.float32).reshape(B, T, C)


# revision 37
# speedup vs baseline: 1.1069x; 1.1069x over previous
"""Trainium2 Bass kernel for nn_MultiHeadAttention (B=4,T=2048,C=1024,H=16,D=64).

Sharding: tensor-parallel over heads. 8 cores x 2 heads each.
Per core: QKV column slices (128 dims), full attention for its 2 heads,
Wo row slice -> bf16 partial output summed on host.

v5 design (v4 baseline measured 667us; trace: ACT exp 285us but first exp
at t=122us and 163us of ACT gaps; PE 232us at half-clock from HAM
oscillation; DVE 345us):
- ACT is pure exp (all PSUM evacuations on DVE): exp stream = 73.4us/batch
  is the metronome; everything else hides around it.
- Prologue: 4 proj chunks on 4 separate PSUM tags (sgA/sgB/ot0/ot1 slots,
  free before attention) -> dense PE, no RoPE ping-pong; V-pass overlaps
  the rsqrt DRAM roundtrip; k-applies before q-applies.
- RoPE: rot-half = one full-width mul by host-PRE-PERMUTED sin, then 4
  cheap bf16 SBUF->SBUF 32-block shift copies (194ns, 4x mode) + GpSimd
  add. Was 4x 686ns partition-sliced muls from PSUM.
- ss-mm / V-transposes / apply-broadcasts moved from sgA/sgB (collided
  with live attention S tiles) to mm512.
- ot tiles are [128,512]-shaped (1 bank; PV writes rows 0:65): row
  slices then behave on DVE ([65,512]-shaped tiles mis-read, probed).
  ytb[64:128] <- ot1[0:64] direct DVE copy (+64 shift probed OK), no
  stg+DMA roundtrip.
- sumexp normalize split: ot evacuation inline at qc end; rcp + bmm
  broadcast (one [128,512] psum, h1 at base-partition 64) + ytb muls
  deferred as the FIRST filler of the next qc -> no qc-boundary stall.
- Attention group order: S(g+1) | PV(g,h0) | filler | PV(g,h1) | filler.
- p3: both oc psums evacuated into one [128,2,512] bf16 ob tile, ONE
  row-contiguous DMA per token tile; out partials bf16 (halves DMA).
- exp table preloaded via dummy activation at t=0.

Probed pitfalls (do not regress): DVE APs cannot have strided or
stride-0 partition dims; reciprocal_approx_* input must start at
partition 0; TRN2 matmul output must be f32; single-DMA 2-block
partition swaps transfer wrong elements (use 4 contiguous DMAs or DVE).
"""
import sys

sys.path.insert(0, "/opt/trn_rl_repo")
import numpy as np
import ml_dtypes

BF16NP = ml_dtypes.bfloat16

N_CORES = 8
B_FULL, T_FULL, C = 4, 2048, 1024
H, D = 16, 64
HPC = H // N_CORES          # heads per core = 2
M2 = HPC * D                # 128
EPS = 1e-6

_NC_CACHE: dict = {}


def build_nc(B: int, T: int):
    import concourse.bass as bass
    import concourse.mybir as mybir
    from concourse import bacc
    from concourse.tile import TileContext

    BF16 = mybir.dt.bfloat16
    F32 = mybir.dt.float32
    AF = mybir.ActivationFunctionType
    ALU = mybir.AluOpType

    TT = B * T
    NCIN = C // 128             # 8 contraction tiles for projections
    CPB = T // 512              # 4 chunks of 512 tokens per batch
    NKT = T // 128              # 16 key tiles per batch
    NQC = T // 512              # 4 q chunks per batch
    NG = NKT // 2               # 8 key groups (KGS=2) per q chunk

    nc = bacc.Bacc("TRN2", target_bir_lowering=False, debug=False,
                   num_devices=N_CORES)

    xT_d = nc.dram_tensor("xT", [128, C // 128, TT], BF16,
                          kind="ExternalInput")
    wq_d = nc.dram_tensor("wq", [128, NCIN, M2], BF16, kind="ExternalInput")
    wk_d = nc.dram_tensor("wk", [128, NCIN, M2], BF16, kind="ExternalInput")
    wv_d = nc.dram_tensor("wv", [128, NCIN, M2], BF16, kind="ExternalInput")
    wo_d = nc.dram_tensor("wo", [M2, C], BF16, kind="ExternalInput")
    cos_d = nc.dram_tensor("cos2", [M2, T], F32, kind="ExternalInput")
    sinp_d = nc.dram_tensor("sin2p", [M2, T], F32, kind="ExternalInput")
    ident_d = nc.dram_tensor("ident", [128, 128], BF16, kind="ExternalInput")
    ones4q_d = nc.dram_tensor("ones4q", [128, 4], BF16, kind="ExternalInput")
    ones4k_d = nc.dram_tensor("ones4k", [128, 4], BF16, kind="ExternalInput")
    sel2_d = nc.dram_tensor("sel2", [2, 128], BF16, kind="ExternalInput")
    ones66_d = nc.dram_tensor("ones66", [128, T // 128, 66], BF16,
                              kind="ExternalInput")
    out_d = nc.dram_tensor("out", [TT, C], BF16, kind="ExternalOutput")

    with TileContext(nc) as tc:
        with (
            tc.tile_pool(name="const", bufs=1) as cp,
            tc.tile_pool(name="big", bufs=2) as bigp,
            tc.tile_pool(name="xs", bufs=4) as xsp,
            tc.tile_pool(name="attn", bufs=2) as atp,
            tc.tile_pool(name="scr", bufs=2) as scp,
            tc.tile_pool(name="drs", bufs=2, space="DRAM") as drp,
            tc.tile_pool(name="ps", bufs=1, space="PSUM") as psp,
        ):
            # exp table preload: tiny dummy activation fires immediately
            warm_in = cp.tile([1, 16], F32, tag="warm_in")
            warm_out = cp.tile([1, 16], BF16, tag="warm_out")
            nc.vector.memset(warm_in, 0.0)
            nc.scalar.activation(warm_out, warm_in, AF.Exp, scale=1.0)

            wq_sb = cp.tile([128, NCIN, M2], BF16, tag="wq")
            wk_sb = cp.tile([128, NCIN, M2], BF16, tag="wk")
            wv_sb = cp.tile([128, NCIN, M2], BF16, tag="wv")
            wo_sb = cp.tile([128, C], BF16, tag="wo")
            cos_sb = cp.tile([128, T], F32, tag="cos")
            sinp_sb = cp.tile([128, T], F32, tag="sinp")
            ident = cp.tile([128, 128], BF16, tag="ident")
            ones4q = cp.tile([128, 4], BF16, tag="ones4q")
            ones4k = cp.tile([128, 4], BF16, tag="ones4k")
            sel2 = cp.tile([2, 128], BF16, tag="sel2")

            # wq/wk first: the first proj MMs need only wq + x chunk 0
            # (x prefetches are emitted right after the consts below)
            nc.sync.dma_start(out=wq_sb, in_=wq_d[:, :, :])
            nc.sync.dma_start(out=wk_sb, in_=wk_d[:, :, :])

            # rotate-half shift: out block <- t block (within-head swap)
            ROT_BLOCKS = (((0, 32), (32, 64)), ((32, 64), (0, 32)),
                          ((64, 96), (96, 128)), ((96, 128), (64, 96)))

            st: dict = {}
            xq: dict = {}

            def get_state(b):
                if b in st:
                    return st[b]
                qtb = bigp.tile([128, T], BF16, tag="qtb")
                ktb = bigp.tile([128, T], BF16, tag="ktb")
                ytb = bigp.tile([128, T], BF16, tag="ytb")
                vsb = bigp.tile([128, NKT, 130], BF16, tag="vsb")
                ssqk = scp.tile([4, T], F32, tag="ssqk", bufs=1)
                sc_q = scp.tile([2, T], BF16, tag="sc_q", bufs=1)
                sc_k = scp.tile([2, T], BF16, tag="sc_k", bufs=1)
                sums_t = drp.tile([4, T], F32, tag="sums")
                scales_t = drp.tile([4, T], BF16, tag="scales")
                if b < 2:
                    # ones columns persist in the physical buffer; later
                    # batches reuse them (V copies never touch cols 64/129)
                    nc.sync.dma_start(out=vsb[:, :, 64:130],
                                      in_=ones66_d[:, :, :])
                s = dict(qtb=qtb, ktb=ktb, ytb=ytb, vsb=vsb, ssqk=ssqk,
                         sc_q=sc_q, sc_k=sc_k, sums_t=sums_t,
                         scales_t=scales_t)
                st[b] = s
                return s

            def prefetch_x(b, c, eng=None):
                """Issue the x-chunk DMA ahead of its consuming quantum."""
                if (b, c) in xq or b >= B or c >= CPB:
                    return
                x_sb = xsp.tile([128, NCIN, 512], BF16, tag="x")
                (eng or nc.sync).dma_start(
                    out=x_sb,
                    in_=xT_d[:, :, b * T + c * 512: b * T + (c + 1) * 512])
                xq[(b, c)] = x_sb

            def rope_emit(s, name, ps, cc, act_sq=False):
                """RoPE for one projected 512-chunk (DVE+GpSimd only).
                Returns the squared tile for the deferred ss matmul.
                act_sq: square on the (idle) ACT engine -- prologue only,
                where the serial GpSimd add+sq chain gates the first exp."""
                dkey = "qtb" if name == "q" else "ktb"
                dd = s[dkey][:, cc]
                nc.vector.tensor_mul(out=dd, in0=ps, in1=cos_sb[:, cc])
                rot_t = scp.tile([128, 512], BF16, tag="rot_t")
                nc.vector.tensor_mul(out=rot_t, in0=ps, in1=sinp_sb[:, cc])
                rots = scp.tile([128, 512], BF16, tag="rots")
                for (d0, d1), (s0, s1) in ROT_BLOCKS:
                    nc.vector.tensor_copy(out=rots[d0:d1], in_=rot_t[s0:s1])
                nc.gpsimd.tensor_add(out=dd, in0=dd, in1=rots)
                # bufs=8: lives until the ss quantum; prologue round-robins
                # 4 chunks x {q,k}
                sq = scp.tile([128, 512], BF16, tag="sq", bufs=8)
                if act_sq:
                    nc.scalar.activation(sq, dd, AF.Square, scale=1.0)
                else:
                    nc.gpsimd.tensor_mul(out=sq, in0=dd, in1=dd)
                return sq

            def p1_qk_gen(b, c, qtag="mm512", ktag="mm512", tbufs=2,
                          mm_step=2, act_sq=False):
                """Q/K projections + RoPE + fused q+k sumsq for one chunk."""
                s = get_state(b)
                cc = slice(c * 512, (c + 1) * 512)
                prefetch_x(b, c)
                x_sb = xq[(b, c)]
                yield
                sqs = {}
                for name, w_sb, tag in (("q", wq_sb, qtag),
                                        ("k", wk_sb, ktag)):
                    ps = psp.tile([128, 512], F32, tag=tag, bufs=tbufs,
                                  name=f"acc_{name}{b}_{c}")
                    for ci in range(NCIN):
                        nc.tensor.matmul(ps, w_sb[:, ci], x_sb[:, ci],
                                         start=(ci == 0), stop=(ci == NCIN - 1),
                                         skip_group_check=True)
                        if ci % mm_step == mm_step - 1 and ci != NCIN - 1:
                            yield
                    sqs[name] = rope_emit(s, name, ps, cc, act_sq=act_sq)
                    yield
                # deferred sumsq: one psum tile, q rows 0:2 / k rows 2:4
                # via zero-padded selectors + psum accumulation. Deferring
                # to its own quantum keeps the PE queue head from stalling
                # on the GpSimd squares.
                ps_ss = psp.tile([4, 512], F32, tag="mm512", bufs=2,
                                 name=f"ss{b}_{c}")
                nc.tensor.matmul(ps_ss, ones4q, sqs["q"], start=True,
                                 stop=False, skip_group_check=True)
                nc.tensor.matmul(ps_ss, ones4k, sqs["k"], start=False,
                                 stop=True, skip_group_check=True)
                nc.vector.tensor_copy(out=s["ssqk"][:, cc], in_=ps_ss)
                if c == CPB - 1:
                    nc.sync.dma_start(out=s["sums_t"][:, :], in_=s["ssqk"])
                yield

            def p1_rsqrt_gen(b):
                """rsqrt of mean-square, DVE-only (magic constant + 2 Newton
                steps; no ACT table switch), then scales roundtrip."""
                import concourse.mybir as mybir
                I32 = mybir.dt.int32
                s = get_state(b)
                FP = 4 * T // 128
                pk = scp.tile([128, 5, FP], F32, tag="pk", bufs=1)
                y1b = scp.tile([128, FP], BF16, tag="y1b", bufs=1)
                nc.sync.dma_start(
                    out=pk[:, 0],
                    in_=s["sums_t"][:].rearrange("a t -> (a t)")
                    .rearrange("(p f) -> p f", p=128))
                ms, g, t1, tmp = (pk[:, j] for j in range(1, 5))
                nc.vector.tensor_scalar(out=ms, in0=pk[:, 0], scalar1=1.0 / D,
                                        scalar2=EPS, op0=ALU.mult,
                                        op1=ALU.add)
                # g0 via 0x5f3759df bit trick, then 2 Newton iterations
                nc.vector.tensor_scalar(
                    out=tmp.bitcast(I32), in0=ms.bitcast(I32), scalar1=1,
                    scalar2=0, op0=ALU.logical_shift_right,
                    op1=ALU.bitwise_or)
                nc.vector.tensor_scalar(
                    out=g.bitcast(I32), in0=tmp.bitcast(I32), scalar1=-1,
                    scalar2=0x5F3759DF, op0=ALU.mult, op1=ALU.add)
                for it in range(2):
                    nc.vector.tensor_mul(out=t1, in0=g, in1=g)
                    nc.vector.tensor_mul(out=t1, in0=t1, in1=ms)
                    nc.vector.tensor_scalar(out=t1, in0=t1, scalar1=-0.5,
                                            scalar2=1.5, op0=ALU.mult,
                                            op1=ALU.add)
                    out_ap = g if it == 0 else y1b
                    nc.vector.tensor_mul(out=out_ap, in0=g, in1=t1)
                nc.sync.dma_start(
                    out=s["scales_t"][:].rearrange("a t -> (a t)")
                    .rearrange("(p f) -> p f", p=128),
                    in_=y1b)
                nc.sync.dma_start(out=s["sc_q"][:, :], in_=s["scales_t"][0:2, :])
                nc.scalar.dma_start(out=s["sc_k"][:, :],
                                    in_=s["scales_t"][2:4, :])
                yield

            def p1_v_gen(b, c, vtag="mm512", tbufs=2, mm_step=2):
                """V projection + transpose into vsb for one chunk."""
                s = get_state(b)
                vsb = s["vsb"]
                x_sb = xq.pop((b, c))
                ps = psp.tile([128, 512], F32, tag=vtag, bufs=tbufs,
                              name=f"acc_v{b}_{c}")
                for ci in range(NCIN):
                    nc.tensor.matmul(ps, wv_sb[:, ci], x_sb[:, ci],
                                     start=(ci == 0), stop=(ci == NCIN - 1),
                                     skip_group_check=True)
                    if ci % mm_step == mm_step - 1 and ci != NCIN - 1:
                        yield
                # bufs=4: the prologue round-robins 4 chunks; with 2 slots
                # vtmp_c2 waits transposes_c0 whose mm512 release sits
                # behind vtmp_c2 in the strict DVE FIFO -> deadlock.
                vtmp = scp.tile([128, 512], BF16, tag="vtmp", bufs=4)
                nc.vector.tensor_copy(out=vtmp, in_=ps)
                yield
                for i in range(4):
                    kt_idx = c * 4 + i
                    ps_t = psp.tile([128, 128], BF16, tag="mm512", bufs=2,
                                    name=f"tp{b}_{kt_idx}")
                    nc.tensor.transpose(ps_t, vtmp[:, i * 128:(i + 1) * 128],
                                        ident)
                    # both head halves in one copy: cols {0:64, 65:129}
                    nc.vector.tensor_copy(
                        out=vsb[:, kt_idx, :]
                        .rearrange("p (h x) -> p h x", h=2)[:, :, 0:64],
                        in_=ps_t[:].rearrange("p (h i) -> p h i", h=2))
                    if i == 1:
                        yield

            def p1_apply_gen(b, order=("k", "q")):
                """Apply norm scales to qtb/ktb via outer-product bcasts.
                K first so attention's S matmuls unblock earlier."""
                s = get_state(b)
                for which in order:
                    dkey, sckey = (("qtb", "sc_q") if which == "q"
                                   else ("ktb", "sc_k"))
                    for c in range(CPB):
                        cc = slice(c * 512, (c + 1) * 512)
                        bps = psp.tile([128, 512], F32, tag="mm512", bufs=2,
                                       name=f"ap{b}{which}{c}")
                        nc.tensor.matmul(bps, sel2, s[sckey][:, cc],
                                         start=True, stop=True,
                                         skip_group_check=True)
                        nc.vector.tensor_mul(out=s[dkey][:, cc],
                                             in0=s[dkey][:, cc], in1=bps)
                        yield

            def p1_chain(b):
                return ([p1_qk_gen(b, c) for c in range(CPB)]
                        + [p1_rsqrt_gen(b)]
                        + [p1_v_gen(b, c) for c in range(CPB)]
                        + [p1_apply_gen(b)])

            def p3_chunk_gen(b, qc):
                """Wo projection + bf16 partial-output DMA, one DMA per tt."""
                s = get_state(b)
                for tt in range(qc * 4, qc * 4 + 4):
                    ob = scp.tile([128, 2, 512], BF16, tag="ob")
                    for oc in range(2):
                        pso = psp.tile([128, 512], F32, tag="mm512", bufs=2,
                                       name=f"wo{b}_{tt}_{oc}")
                        nc.tensor.matmul(
                            pso, s["ytb"][:, tt * 128:(tt + 1) * 128],
                            wo_sb[:, oc * 512:(oc + 1) * 512],
                            start=True, stop=True, skip_group_check=True)
                        nc.vector.tensor_copy(out=ob[:, oc], in_=pso)
                        if oc == 0:
                            yield
                    nc.sync.dma_start(
                        out=out_d[b * T + tt * 128: b * T + (tt + 1) * 128, :],
                        in_=ob.rearrange("p a f -> p (a f)"))
                    yield

            def norm_gen(b, qc, sml):
                """Deferred sumexp-normalize: rcp (bf16 staged) + one
                broadcast matmul (h1 at base partition 64) + 2 ytb muls."""
                s = get_state(b)
                qq = slice(qc * 512, (qc + 1) * 512)
                rcpf = scp.tile([1, 2, 512], F32, tag="rcpf")
                nc.vector.reciprocal_approx_fast(out=rcpf[:], in_=sml[:])
                rcpb = scp.tile([1, 2, 512], BF16, tag="rcpb")
                # single-partition SBUF->SBUF convert: GpSimd core 0, off
                # the loaded DVE
                nc.gpsimd.tensor_scalar(out=rcpb, in0=rcpf, scalar1=1.0,
                                        scalar2=0.0, op0=ALU.mult,
                                        op1=ALU.add)
                bps = psp.tile([128, 512], F32, tag="mm512", bufs=2,
                               name=f"nb{b}_{qc}")
                for h in range(2):
                    # sel2 row 0, cols 0:64 is an all-ones [1,64] selector;
                    # bf16 rhs keeps the MM at 1 cyc/row (f32 rhs is 4x)
                    nc.tensor.matmul(bps[h * 64:(h + 1) * 64],
                                     sel2[0:1, 0:64],
                                     rcpb[:, h], start=True, stop=True,
                                     skip_group_check=True)
                yield
                # bps rows 0:64 / 64:128 hold the per-head reciprocal
                # broadcasts -> one full-width mul normalizes both heads
                nc.vector.tensor_mul(out=s["ytb"][:, qq],
                                     in0=s["ytb"][:, qq], in1=bps)
                yield

            def attn_qc(b, qc, fillers, prev_tail):
                """Attention for one 512-query chunk; KGS=2, heads on
                separate PSUM tags, pipelined by one key-group.

                The first group's S matmuls are emitted BEFORE the previous
                qc's ot-evacuation (prev_tail) so the exp stream never gaps
                at the qc boundary. Returns this qc's tail closure, which
                evacuates ot and queues [norm, p3] as a filler chain."""
                s = get_state(b)
                qtb, ktb, vsb, ytb = s["qtb"], s["ktb"], s["vsb"], s["ytb"]
                qq = slice(qc * 512, (qc + 1) * 512)
                # [128,512]-shaped tiles; PV writes rows 0:65 only
                ot = [psp.tile([128, 512], F32, tag=f"ot{h}", bufs=1,
                               name=f"ot{h}_{b}_{qc}")
                      for h in range(2)]

                def emit_sg(g):
                    """S for both heads, kt-major MM order: consecutive
                    (kt,h0)/(kt,h1) MMs contract over disjoint PE row
                    groups (rows 0:64 vs 64:128 -> tile_position auto-
                    derived from base partitions), so each pair runs
                    CONCURRENTLY in the array -- S costs ~2 MM slots,
                    not 4."""
                    sps = [psp.tile([128, 2, 512], F32,
                                    tag=("sgA" if h == 0 else "sgB"),
                                    bufs=1, name=f"sg{h}")
                           for h in range(2)]
                    for i in range(2):
                        ktg = g * 2 + i
                        for h in range(2):
                            hs = slice(h * 64, (h + 1) * 64)
                            nc.tensor.matmul(
                                sps[h][:, i],
                                ktb[hs, ktg * 128:(ktg + 1) * 128],
                                qtb[hs, qq],
                                start=True, stop=True,
                                skip_group_check=True)
                    stexps = []
                    for h in range(2):
                        stexp = atp.tile([128, 2, 512], BF16,
                                         tag=("seA" if h == 0 else "seB"),
                                         name=f"se{h}")
                        nc.scalar.activation(stexp, sps[h], AF.Exp,
                                             scale=0.125)
                        stexps.append(stexp)
                    return stexps

                def advance_filler():
                    while fillers:
                        chain = fillers[0]
                        try:
                            next(chain[0])
                            if len(fillers) > 1:
                                fillers.append(fillers.pop(0))
                            break
                        except StopIteration:
                            chain.pop(0)
                            if not chain:
                                fillers.pop(0)

                def pv(g, h, pend):
                    for i in range(2):
                        ktg = g * 2 + i
                        nc.tensor.matmul(
                            ot[h][0:65], vsb[:, ktg, h * 65:h * 65 + 65],
                            pend[h][:, i],
                            start=(ktg == 0), stop=(ktg == NKT - 1),
                            skip_group_check=True)

                # Pipelined by one group. Fillers go BEFORE each head's
                # S(g+1): both S(g+1,h) and PV(g,h) stall on exp(h,g)
                # completion (sgX slot release / stexp ready), so the
                # filler quantum absorbs that sem-latency window instead
                # of the PE queue head exposing it.
                pend = emit_sg(0)
                if prev_tail is not None:
                    fillers.insert(0, prev_tail())
                for g in range(NG):
                    advance_filler()
                    nxt = emit_sg(g + 1) if g + 1 < NG else None
                    pv(g, 0, pend)
                    advance_filler()
                    pv(g, 1, pend)
                    advance_filler()
                    pend = nxt

                def tail():
                    # evacuate ot: sumexp rows + O rows (DVE only), then
                    # queue the deferred normalize + Wo chain as filler
                    sml = scp.tile([1, 2, 512], F32, tag="sml")
                    nc.vector.tensor_copy(out=sml[:, 0], in_=ot[0][64:65])
                    nc.vector.tensor_copy(out=sml[:, 1], in_=ot[1][64:65])
                    nc.vector.tensor_copy(out=ytb[0:64, qq], in_=ot[0][0:64])
                    nc.vector.tensor_copy(out=ytb[64:128, qq],
                                          in_=ot[1][0:64])
                    return [norm_gen(b, qc, sml), p3_chunk_gen(b, qc)]

                return tail

            # ---- prologue: batch 0 p1, dense PE via 4 separate psum tags
            PRO_TAGS = ("sgA", "sgB", "ot0", "ot1")

            def rr(gens):
                live = list(gens)
                while live:
                    for g in list(live):
                        try:
                            next(g)
                        except StopIteration:
                            live.remove(g)

            # cos/sin before x: the first RoPE needs them; x chunks spread
            # over both HWDGE queues (SP + ACT) to avoid serializing
            nc.sync.dma_start(out=cos_sb, in_=cos_d[:, :])
            nc.scalar.dma_start(out=sinp_sb, in_=sinp_d[:, :])
            for c in range(CPB):
                prefetch_x(0, c, eng=(nc.scalar if c % 2 else nc.sync))
            nc.scalar.dma_start(out=ones4q, in_=ones4q_d[:, :])
            nc.scalar.dma_start(out=ones4k, in_=ones4k_d[:, :])
            nc.sync.dma_start(out=wv_sb, in_=wv_d[:, :, :])
            nc.scalar.dma_start(out=ident, in_=ident_d[:, :])
            nc.scalar.dma_start(out=sel2, in_=sel2_d[:, :])
            nc.sync.dma_start(out=wo_sb, in_=wo_d[:, :])
            # pairs (not 4-wide): chunk 0's RoPE/squares chain starts ~2x
            # earlier, which gates the scales roundtrip -> first exp
            for pair in ((0, 1), (2, 3)):
                rr([p1_qk_gen(0, c, qtag=PRO_TAGS[c], ktag=PRO_TAGS[c],
                              tbufs=1, mm_step=4, act_sq=True)
                    for c in pair])
            rq = p1_rsqrt_gen(0)
            for _ in rq:
                pass
            for pair in ((0, 1), (2, 3)):
                rr([p1_v_gen(0, c, vtag=PRO_TAGS[c], tbufs=1, mm_step=4)
                    for c in pair])
            # apply LAST: its DVE muls wait on the scales DMA roundtrip;
            # emitting them earlier blocks the strict DVE FIFO (and its
            # mm512 bps allocs deadlock against the V transposes).
            for _ in p1_apply_gen(0):
                pass

            # ---- steady state ----
            tail = None
            for b in range(B):
                fillers = []
                if b + 1 < B:
                    fillers.append(p1_chain(b + 1))
                for qc in range(NQC):
                    if b + 1 < B and qc < CPB:
                        prefetch_x(b + 1, qc)
                    tail = attn_qc(b, qc, fillers, tail)
                # drain leftover filler quanta at the batch boundary
                for g in fillers:
                    for gen in g:
                        for _ in gen:
                            pass
                fillers.clear()
            # final qc's evacuation + norm + Wo
            if tail is not None:
                for gen in tail():
                    for _ in gen:
                        pass

    nc.compile()
    return nc


def make_core_inputs(x, cos, sin, Wq, Wk, Wv, Wo, B, T):
    """Host-side sharding. Returns list of 8 input dicts."""
    TT = B * T
    NCIN = C // 128
    # [128 ci, NCIN co, TT] so each x-chunk DMA reads contiguous rows
    xT = np.ascontiguousarray(
        np.asarray(x, np.float32).reshape(TT, C).T.astype(BF16NP)
        .reshape(NCIN, 128, TT).transpose(1, 0, 2))
    cosT = np.asarray(cos, np.float32).reshape(T, D).T      # [64, T]
    sinT = np.asarray(sin, np.float32).reshape(T, D).T
    sin_signed = np.concatenate([-sinT[0:32], sinT[32:64]], axis=0)
    # pre-permuted: sin2p[p] = sin_signed[partner(p)] (32-block swap)
    sin_perm = np.concatenate([sin_signed[32:64], sin_signed[0:32]], axis=0)
    cos2 = np.ascontiguousarray(np.concatenate([cosT, cosT], axis=0))
    sin2p = np.ascontiguousarray(np.concatenate([sin_perm, sin_perm],
                                                axis=0))
    ones4q = np.zeros((128, 4), BF16NP)
    ones4q[0:64, 0] = 1.0
    ones4q[64:128, 1] = 1.0
    ones4k = np.zeros((128, 4), BF16NP)
    ones4k[0:64, 2] = 1.0
    ones4k[64:128, 3] = 1.0
    sel2 = np.zeros((2, 128), BF16NP)
    sel2[0, 0:64] = 1.0
    sel2[1, 64:128] = 1.0

    def wprep(W, rows):
        # [128 ci, NCIN co, M2] so the weight DMA is contiguous
        wT = np.asarray(W, np.float32)[rows].T.astype(BF16NP)   # [C, M2]
        return np.ascontiguousarray(
            wT.reshape(NCIN, 128, M2).transpose(1, 0, 2))

    in_maps = []
    for core in range(N_CORES):
        rows = slice(core * M2, (core + 1) * M2)
        in_maps.append({
            "xT": xT,
            "wq": wprep(Wq, rows),
            "wk": wprep(Wk, rows),
            "wv": wprep(Wv, rows),
            "wo": np.ascontiguousarray(
                np.asarray(Wo, np.float32)[:, rows].T.astype(BF16NP)),
            "cos2": cos2,
            "sin2p": sin2p,
            "ident": np.eye(128, dtype=BF16NP),
            "ones4q": ones4q,
            "ones4k": ones4k,
            "sel2": sel2,
            "ones66": np.ones((128, T // 128, 66), BF16NP),
        })
    return in_maps


def kernel(x, cos, sin, Wq, Wk, Wv, Wo):
    from concourse.bass_utils import run_bass_kernel_spmd

    B, T = x.shape[0], x.shape[1]
    key = (B, T)
    if key not in _NC_CACHE:
        _NC_CACHE[key] = build_nc(B, T)
    nc = _NC_CACHE[key]
    in_maps = make_core_inputs(x, cos, sin, Wq, Wk, Wv, Wo, B, T)
    res = run_bass_kernel_spmd(nc, in_maps, core_ids=list(range(N_CORES)))
    out = np.zeros((B * T, C), np.float64)
    for r in res.results:
        out += r["out"].astype(np.float64)
    return out.astype(np.float32).reshape(B, T, C)


# revision 38
# speedup vs baseline: 1.1553x; 1.0437x over previous
"""Trainium2 Bass kernel for nn_MultiHeadAttention (B=4,T=2048,C=1024,H=16,D=64).

Sharding: tensor-parallel over heads. 8 cores x 2 heads each.
Per core: QKV column slices (128 dims), full attention for its 2 heads,
Wo row slice -> bf16 partial output summed on host.

v5 design (v4 baseline measured 667us; trace: ACT exp 285us but first exp
at t=122us and 163us of ACT gaps; PE 232us at half-clock from HAM
oscillation; DVE 345us):
- ACT is pure exp (all PSUM evacuations on DVE): exp stream = 73.4us/batch
  is the metronome; everything else hides around it.
- Prologue: 4 proj chunks on 4 separate PSUM tags (sgA/sgB/ot0/ot1 slots,
  free before attention) -> dense PE, no RoPE ping-pong; V-pass overlaps
  the rsqrt DRAM roundtrip; k-applies before q-applies.
- RoPE: rot-half = one full-width mul by host-PRE-PERMUTED sin, then 4
  cheap bf16 SBUF->SBUF 32-block shift copies (194ns, 4x mode) + GpSimd
  add. Was 4x 686ns partition-sliced muls from PSUM.
- ss-mm / V-transposes / apply-broadcasts moved from sgA/sgB (collided
  with live attention S tiles) to mm512.
- ot tiles are [128,512]-shaped (1 bank; PV writes rows 0:65): row
  slices then behave on DVE ([65,512]-shaped tiles mis-read, probed).
  ytb[64:128] <- ot1[0:64] direct DVE copy (+64 shift probed OK), no
  stg+DMA roundtrip.
- sumexp normalize split: ot evacuation inline at qc end; rcp + bmm
  broadcast (one [128,512] psum, h1 at base-partition 64) + ytb muls
  deferred as the FIRST filler of the next qc -> no qc-boundary stall.
- Attention group order: S(g+1) | PV(g,h0) | filler | PV(g,h1) | filler.
- p3: both oc psums evacuated into one [128,2,512] bf16 ob tile, ONE
  row-contiguous DMA per token tile; out partials bf16 (halves DMA).
- exp table preloaded via dummy activation at t=0.

Probed pitfalls (do not regress): DVE APs cannot have strided or
stride-0 partition dims; reciprocal_approx_* input must start at
partition 0; TRN2 matmul output must be f32; single-DMA 2-block
partition swaps transfer wrong elements (use 4 contiguous DMAs or DVE).
"""
import sys

sys.path.insert(0, "/opt/trn_rl_repo")
import numpy as np
import ml_dtypes

BF16NP = ml_dtypes.bfloat16

N_CORES = 8
B_FULL, T_FULL, C = 4, 2048, 1024
H, D = 16, 64
HPC = H // N_CORES          # heads per core = 2
M2 = HPC * D                # 128
EPS = 1e-6

_NC_CACHE: dict = {}


def build_nc(B: int, T: int):
    import concourse.bass as bass
    import concourse.mybir as mybir
    from concourse import bacc
    from concourse.tile import TileContext

    BF16 = mybir.dt.bfloat16
    F32 = mybir.dt.float32
    AF = mybir.ActivationFunctionType
    ALU = mybir.AluOpType

    TT = B * T
    NCIN = C // 128             # 8 contraction tiles for projections
    CPB = T // 512              # 4 chunks of 512 tokens per batch
    NKT = T // 128              # 16 key tiles per batch
    NQC = T // 512              # 4 q chunks per batch
    NG = NKT // 2               # 8 key groups (KGS=2) per q chunk

    nc = bacc.Bacc("TRN2", target_bir_lowering=False, debug=False,
                   num_devices=N_CORES)

    xT_d = nc.dram_tensor("xT", [128, C // 128, TT], BF16,
                          kind="ExternalInput")
    wq_d = nc.dram_tensor("wq", [128, NCIN, M2], BF16, kind="ExternalInput")
    wk_d = nc.dram_tensor("wk", [128, NCIN, M2], BF16, kind="ExternalInput")
    wv_d = nc.dram_tensor("wv", [128, NCIN, M2], BF16, kind="ExternalInput")
    wo_d = nc.dram_tensor("wo", [M2, C], BF16, kind="ExternalInput")
    cos_d = nc.dram_tensor("cos2", [M2, T], F32, kind="ExternalInput")
    sinp_d = nc.dram_tensor("sin2p", [M2, T], F32, kind="ExternalInput")
    ident_d = nc.dram_tensor("ident", [128, 128], BF16, kind="ExternalInput")
    ones4q_d = nc.dram_tensor("ones4q", [128, 4], BF16, kind="ExternalInput")
    ones4k_d = nc.dram_tensor("ones4k", [128, 4], BF16, kind="ExternalInput")
    sel2_d = nc.dram_tensor("sel2", [2, 128], BF16, kind="ExternalInput")
    ones66_d = nc.dram_tensor("ones66", [128, T // 128, 66], BF16,
                              kind="ExternalInput")
    out_d = nc.dram_tensor("out", [TT, C], BF16, kind="ExternalOutput")

    with TileContext(nc) as tc:
        with (
            tc.tile_pool(name="const", bufs=1) as cp,
            tc.tile_pool(name="big", bufs=2) as bigp,
            tc.tile_pool(name="xs", bufs=4) as xsp,
            tc.tile_pool(name="attn", bufs=2) as atp,
            tc.tile_pool(name="scr", bufs=2) as scp,
            tc.tile_pool(name="drs", bufs=2, space="DRAM") as drp,
            tc.tile_pool(name="ps", bufs=1, space="PSUM") as psp,
        ):
            # exp table preload: tiny dummy activation fires immediately
            warm_in = cp.tile([1, 16], F32, tag="warm_in")
            warm_out = cp.tile([1, 16], BF16, tag="warm_out")
            nc.vector.memset(warm_in, 0.0)
            nc.scalar.activation(warm_out, warm_in, AF.Exp, scale=1.0)

            wq_sb = cp.tile([128, NCIN, M2], BF16, tag="wq")
            wk_sb = cp.tile([128, NCIN, M2], BF16, tag="wk")
            wv_sb = cp.tile([128, NCIN, M2], BF16, tag="wv")
            wo_sb = cp.tile([128, C], BF16, tag="wo")
            cos_sb = cp.tile([128, T], F32, tag="cos")
            sinp_sb = cp.tile([128, T], F32, tag="sinp")
            ident = cp.tile([128, 128], BF16, tag="ident")
            ones4q = cp.tile([128, 4], BF16, tag="ones4q")
            ones4k = cp.tile([128, 4], BF16, tag="ones4k")
            sel2 = cp.tile([2, 128], BF16, tag="sel2")

            # wq/wk first: the first proj MMs need only wq + x chunk 0
            # (x prefetches are emitted right after the consts below)
            nc.sync.dma_start(out=wq_sb, in_=wq_d[:, :, :])
            nc.sync.dma_start(out=wk_sb, in_=wk_d[:, :, :])

            # rotate-half shift: out block <- t block (within-head swap)
            ROT_BLOCKS = (((0, 32), (32, 64)), ((32, 64), (0, 32)),
                          ((64, 96), (96, 128)), ((96, 128), (64, 96)))

            st: dict = {}
            xq: dict = {}

            def get_state(b):
                if b in st:
                    return st[b]
                qtb = bigp.tile([128, T], BF16, tag="qtb")
                ktb = bigp.tile([128, T], BF16, tag="ktb")
                ytb = bigp.tile([128, T], BF16, tag="ytb")
                vsb = bigp.tile([128, NKT, 130], BF16, tag="vsb")
                ssqk = scp.tile([4, T], F32, tag="ssqk", bufs=1)
                sc_q = scp.tile([2, T], BF16, tag="sc_q", bufs=1)
                sc_k = scp.tile([2, T], BF16, tag="sc_k", bufs=1)
                sums_t = drp.tile([4, T], F32, tag="sums")
                scales_t = drp.tile([4, T], BF16, tag="scales")
                if b < 2:
                    # ones columns persist in the physical buffer; later
                    # batches reuse them (V copies never touch cols 64/129)
                    nc.sync.dma_start(out=vsb[:, :, 64:130],
                                      in_=ones66_d[:, :, :])
                s = dict(qtb=qtb, ktb=ktb, ytb=ytb, vsb=vsb, ssqk=ssqk,
                         sc_q=sc_q, sc_k=sc_k, sums_t=sums_t,
                         scales_t=scales_t)
                st[b] = s
                return s

            def prefetch_x(b, c, eng=None):
                """Issue the x-chunk DMA ahead of its consuming quantum."""
                if (b, c) in xq or b >= B or c >= CPB:
                    return
                x_sb = xsp.tile([128, NCIN, 512], BF16, tag="x")
                (eng or nc.sync).dma_start(
                    out=x_sb,
                    in_=xT_d[:, :, b * T + c * 512: b * T + (c + 1) * 512])
                xq[(b, c)] = x_sb

            def rope_emit(s, name, ps, cc, act_sq=False):
                """RoPE for one projected 512-chunk (DVE+GpSimd only).
                Returns the squared tile for the deferred ss matmul.
                act_sq: square on the (idle) ACT engine -- prologue only,
                where the serial GpSimd add+sq chain gates the first exp."""
                dkey = "qtb" if name == "q" else "ktb"
                dd = s[dkey][:, cc]
                nc.vector.tensor_mul(out=dd, in0=ps, in1=cos_sb[:, cc])
                rot_t = scp.tile([128, 512], BF16, tag="rot_t")
                nc.vector.tensor_mul(out=rot_t, in0=ps, in1=sinp_sb[:, cc])
                rots = scp.tile([128, 512], BF16, tag="rots")
                for (d0, d1), (s0, s1) in ROT_BLOCKS:
                    nc.vector.tensor_copy(out=rots[d0:d1], in_=rot_t[s0:s1])
                nc.gpsimd.tensor_add(out=dd, in0=dd, in1=rots)
                # bufs=8: lives until the ss quantum; prologue round-robins
                # 4 chunks x {q,k}
                sq = scp.tile([128, 512], BF16, tag="sq", bufs=8)
                if act_sq:
                    nc.scalar.activation(sq, dd, AF.Square, scale=1.0)
                else:
                    nc.gpsimd.tensor_mul(out=sq, in0=dd, in1=dd)
                return sq

            def p1_qk_gen(b, c, qtag="mm512", ktag="mm512", tbufs=2,
                          mm_step=2, act_sq=False):
                """Q/K projections + RoPE + fused q+k sumsq for one chunk."""
                s = get_state(b)
                cc = slice(c * 512, (c + 1) * 512)
                prefetch_x(b, c)
                x_sb = xq[(b, c)]
                yield
                sqs = {}
                for name, w_sb, tag in (("q", wq_sb, qtag),
                                        ("k", wk_sb, ktag)):
                    ps = psp.tile([128, 512], F32, tag=tag, bufs=tbufs,
                                  name=f"acc_{name}{b}_{c}")
                    for ci in range(NCIN):
                        nc.tensor.matmul(ps, w_sb[:, ci], x_sb[:, ci],
                                         start=(ci == 0), stop=(ci == NCIN - 1),
                                         skip_group_check=True)
                        if ci % mm_step == mm_step - 1 and ci != NCIN - 1:
                            yield
                    sqs[name] = rope_emit(s, name, ps, cc, act_sq=act_sq)
                    yield
                # deferred sumsq: one psum tile, q rows 0:2 / k rows 2:4
                # via zero-padded selectors + psum accumulation. Deferring
                # to its own quantum keeps the PE queue head from stalling
                # on the GpSimd squares.
                ps_ss = psp.tile([4, 512], F32, tag="mm512", bufs=2,
                                 name=f"ss{b}_{c}")
                nc.tensor.matmul(ps_ss, ones4q, sqs["q"], start=True,
                                 stop=False, skip_group_check=True)
                nc.tensor.matmul(ps_ss, ones4k, sqs["k"], start=False,
                                 stop=True, skip_group_check=True)
                nc.vector.tensor_copy(out=s["ssqk"][:, cc], in_=ps_ss)
                if c == CPB - 1:
                    nc.sync.dma_start(out=s["sums_t"][:, :], in_=s["ssqk"])
                yield

            def p1_rsqrt_gen(b):
                """rsqrt of mean-square, DVE-only (magic constant + 2 Newton
                steps; no ACT table switch), then scales roundtrip."""
                import concourse.mybir as mybir
                I32 = mybir.dt.int32
                s = get_state(b)
                FP = 4 * T // 128
                pk = scp.tile([128, 5, FP], F32, tag="pk", bufs=1)
                y1b = scp.tile([128, FP], BF16, tag="y1b", bufs=1)
                nc.sync.dma_start(
                    out=pk[:, 0],
                    in_=s["sums_t"][:].rearrange("a t -> (a t)")
                    .rearrange("(p f) -> p f", p=128))
                ms, g, t1, tmp = (pk[:, j] for j in range(1, 5))
                nc.vector.tensor_scalar(out=ms, in0=pk[:, 0], scalar1=1.0 / D,
                                        scalar2=EPS, op0=ALU.mult,
                                        op1=ALU.add)
                # g0 via 0x5f3759df bit trick, then 2 Newton iterations
                nc.vector.tensor_scalar(
                    out=tmp.bitcast(I32), in0=ms.bitcast(I32), scalar1=1,
                    scalar2=0, op0=ALU.logical_shift_right,
                    op1=ALU.bitwise_or)
                nc.vector.tensor_scalar(
                    out=g.bitcast(I32), in0=tmp.bitcast(I32), scalar1=-1,
                    scalar2=0x5F3759DF, op0=ALU.mult, op1=ALU.add)
                for it in range(2):
                    nc.vector.tensor_mul(out=t1, in0=g, in1=g)
                    nc.vector.tensor_mul(out=t1, in0=t1, in1=ms)
                    nc.vector.tensor_scalar(out=t1, in0=t1, scalar1=-0.5,
                                            scalar2=1.5, op0=ALU.mult,
                                            op1=ALU.add)
                    out_ap = g if it == 0 else y1b
                    nc.vector.tensor_mul(out=out_ap, in0=g, in1=t1)
                nc.sync.dma_start(
                    out=s["scales_t"][:].rearrange("a t -> (a t)")
                    .rearrange("(p f) -> p f", p=128),
                    in_=y1b)
                nc.sync.dma_start(out=s["sc_q"][:, :], in_=s["scales_t"][0:2, :])
                nc.scalar.dma_start(out=s["sc_k"][:, :],
                                    in_=s["scales_t"][2:4, :])
                yield

            def p1_v_gen(b, c, vtag="mm512", tbufs=2, mm_step=2):
                """V projection + transpose into vsb for one chunk."""
                s = get_state(b)
                vsb = s["vsb"]
                x_sb = xq.pop((b, c))
                ps = psp.tile([128, 512], F32, tag=vtag, bufs=tbufs,
                              name=f"acc_v{b}_{c}")
                for ci in range(NCIN):
                    nc.tensor.matmul(ps, wv_sb[:, ci], x_sb[:, ci],
                                     start=(ci == 0), stop=(ci == NCIN - 1),
                                     skip_group_check=True)
                    if ci % mm_step == mm_step - 1 and ci != NCIN - 1:
                        yield
                # bufs=4: the prologue round-robins 4 chunks; with 2 slots
                # vtmp_c2 waits transposes_c0 whose mm512 release sits
                # behind vtmp_c2 in the strict DVE FIFO -> deadlock.
                vtmp = scp.tile([128, 512], BF16, tag="vtmp", bufs=4)
                nc.vector.tensor_copy(out=vtmp, in_=ps)
                yield
                for i in range(4):
                    kt_idx = c * 4 + i
                    ps_t = psp.tile([128, 128], BF16, tag="mm512", bufs=2,
                                    name=f"tp{b}_{kt_idx}")
                    nc.tensor.transpose(ps_t, vtmp[:, i * 128:(i + 1) * 128],
                                        ident)
                    # both head halves in one copy: cols {0:64, 65:129}
                    nc.vector.tensor_copy(
                        out=vsb[:, kt_idx, :]
                        .rearrange("p (h x) -> p h x", h=2)[:, :, 0:64],
                        in_=ps_t[:].rearrange("p (h i) -> p h i", h=2))
                    if i == 1:
                        yield

            def p1_apply_gen(b, order=("k", "q")):
                """Apply norm scales to qtb/ktb via outer-product bcasts.
                K first so attention's S matmuls unblock earlier."""
                s = get_state(b)
                for which in order:
                    dkey, sckey = (("qtb", "sc_q") if which == "q"
                                   else ("ktb", "sc_k"))
                    for c in range(CPB):
                        cc = slice(c * 512, (c + 1) * 512)
                        bps = psp.tile([128, 512], F32, tag="mm512", bufs=2,
                                       name=f"ap{b}{which}{c}")
                        nc.tensor.matmul(bps, sel2, s[sckey][:, cc],
                                         start=True, stop=True,
                                         skip_group_check=True)
                        nc.vector.tensor_mul(out=s[dkey][:, cc],
                                             in0=s[dkey][:, cc], in1=bps)
                        yield

            def p1_chain(b):
                return ([p1_qk_gen(b, c) for c in range(CPB)]
                        + [p1_rsqrt_gen(b)]
                        + [p1_v_gen(b, c) for c in range(CPB)]
                        + [p1_apply_gen(b)])

            def p3_chunk_gen(b, qc):
                """Wo projection + bf16 partial-output DMA, one DMA per tt."""
                s = get_state(b)
                for tt in range(qc * 4, qc * 4 + 4):
                    ob = scp.tile([128, 2, 512], BF16, tag="ob")
                    for oc in range(2):
                        pso = psp.tile([128, 512], F32, tag="mm512", bufs=2,
                                       name=f"wo{b}_{tt}_{oc}")
                        nc.tensor.matmul(
                            pso, s["ytb"][:, tt * 128:(tt + 1) * 128],
                            wo_sb[:, oc * 512:(oc + 1) * 512],
                            start=True, stop=True, skip_group_check=True)
                        nc.vector.tensor_copy(out=ob[:, oc], in_=pso)
                        if oc == 0:
                            yield
                    nc.sync.dma_start(
                        out=out_d[b * T + tt * 128: b * T + (tt + 1) * 128, :],
                        in_=ob.rearrange("p a f -> p (a f)"))
                    yield

            def norm_gen(b, qc, sml):
                """Deferred sumexp-normalize: rcp (bf16 staged) + one
                broadcast matmul (h1 at base partition 64) + 2 ytb muls."""
                s = get_state(b)
                qq = slice(qc * 512, (qc + 1) * 512)
                rcpf = scp.tile([1, 2, 512], F32, tag="rcpf")
                nc.vector.reciprocal_approx_fast(out=rcpf[:], in_=sml[:])
                rcpb = scp.tile([1, 2, 512], BF16, tag="rcpb")
                # single-partition SBUF->SBUF convert: GpSimd core 0, off
                # the loaded DVE
                nc.gpsimd.tensor_scalar(out=rcpb, in0=rcpf, scalar1=1.0,
                                        scalar2=0.0, op0=ALU.mult,
                                        op1=ALU.add)
                bps = psp.tile([128, 512], F32, tag="mm512", bufs=2,
                               name=f"nb{b}_{qc}")
                for h in range(2):
                    # sel2 row 0, cols 0:64 is an all-ones [1,64] selector;
                    # bf16 rhs keeps the MM at 1 cyc/row (f32 rhs is 4x)
                    nc.tensor.matmul(bps[h * 64:(h + 1) * 64],
                                     sel2[0:1, 0:64],
                                     rcpb[:, h], start=True, stop=True,
                                     skip_group_check=True)
                yield
                # bps rows 0:64 / 64:128 hold the per-head reciprocal
                # broadcasts -> one full-width mul normalizes both heads
                nc.vector.tensor_mul(out=s["ytb"][:, qq],
                                     in0=s["ytb"][:, qq], in1=bps)
                yield

            def attn_qc(b, qc, fillers, prev_tail):
                """Attention for one 512-query chunk; KGS=2, heads on
                separate PSUM tags, pipelined by one key-group.

                The first group's S matmuls are emitted BEFORE the previous
                qc's ot-evacuation (prev_tail) so the exp stream never gaps
                at the qc boundary. Returns this qc's tail closure, which
                evacuates ot and queues [norm, p3] as a filler chain."""
                s = get_state(b)
                qtb, ktb, vsb, ytb = s["qtb"], s["ktb"], s["vsb"], s["ytb"]
                qq = slice(qc * 512, (qc + 1) * 512)
                # [128,512]-shaped tiles; PV writes rows 0:65 only
                ot = [psp.tile([128, 512], F32, tag=f"ot{h}", bufs=1,
                               name=f"ot{h}_{b}_{qc}")
                      for h in range(2)]

                def emit_sg(g):
                    """S for both heads, kt-major MM order: consecutive
                    (kt,h0)/(kt,h1) MMs contract over disjoint PE row
                    groups (rows 0:64 vs 64:128 -> tile_position auto-
                    derived from base partitions), so each pair runs
                    CONCURRENTLY in the array -- S costs ~2 MM slots,
                    not 4."""
                    sps = [psp.tile([128, 2, 512], F32,
                                    tag=("sgA" if h == 0 else "sgB"),
                                    bufs=1, name=f"sg{h}")
                           for h in range(2)]
                    for i in range(2):
                        ktg = g * 2 + i
                        for h in range(2):
                            hs = slice(h * 64, (h + 1) * 64)
                            nc.tensor.matmul(
                                sps[h][:, i],
                                ktb[hs, ktg * 128:(ktg + 1) * 128],
                                qtb[hs, qq],
                                start=True, stop=True,
                                skip_group_check=True)
                    stexps = []
                    for h in range(2):
                        stexp = atp.tile([128, 2, 512], BF16,
                                         tag=("seA" if h == 0 else "seB"),
                                         name=f"se{h}")
                        nc.scalar.activation(stexp, sps[h], AF.Exp,
                                             scale=0.125)
                        stexps.append(stexp)
                    return stexps

                def advance_filler():
                    while fillers:
                        chain = fillers[0]
                        try:
                            next(chain[0])
                            if len(fillers) > 1:
                                fillers.append(fillers.pop(0))
                            break
                        except StopIteration:
                            chain.pop(0)
                            if not chain:
                                fillers.pop(0)

                def pv(g, h, pend):
                    for i in range(2):
                        ktg = g * 2 + i
                        nc.tensor.matmul(
                            ot[h][0:65], vsb[:, ktg, h * 65:h * 65 + 65],
                            pend[h][:, i],
                            start=(ktg == 0), stop=(ktg == NKT - 1),
                            skip_group_check=True)

                # Pipelined by one group. Fillers go BEFORE each head's
                # S(g+1): both S(g+1,h) and PV(g,h) stall on exp(h,g)
                # completion (sgX slot release / stexp ready), so the
                # filler quantum absorbs that sem-latency window instead
                # of the PE queue head exposing it.
                pend = emit_sg(0)
                if prev_tail is not None:
                    fillers.insert(0, prev_tail())
                for g in range(NG):
                    advance_filler()
                    nxt = emit_sg(g + 1) if g + 1 < NG else None
                    pv(g, 0, pend)
                    advance_filler()
                    pv(g, 1, pend)
                    advance_filler()
                    advance_filler()
                    pend = nxt

                def tail():
                    # evacuate ot: sumexp rows + O rows (DVE only), then
                    # queue the deferred normalize + Wo chain as filler
                    sml = scp.tile([1, 2, 512], F32, tag="sml")
                    nc.vector.tensor_copy(out=sml[:, 0], in_=ot[0][64:65])
                    nc.vector.tensor_copy(out=sml[:, 1], in_=ot[1][64:65])
                    nc.vector.tensor_copy(out=ytb[0:64, qq], in_=ot[0][0:64])
                    nc.vector.tensor_copy(out=ytb[64:128, qq],
                                          in_=ot[1][0:64])
                    return [norm_gen(b, qc, sml), p3_chunk_gen(b, qc)]

                return tail

            # ---- prologue: batch 0 p1, dense PE via 4 separate psum tags
            PRO_TAGS = ("sgA", "sgB", "ot0", "ot1")

            def rr(gens):
                live = list(gens)
                while live:
                    for g in list(live):
                        try:
                            next(g)
                        except StopIteration:
                            live.remove(g)

            # cos/sin before x: the first RoPE needs them; x chunks spread
            # over both HWDGE queues (SP + ACT) to avoid serializing
            nc.sync.dma_start(out=cos_sb, in_=cos_d[:, :])
            nc.scalar.dma_start(out=sinp_sb, in_=sinp_d[:, :])
            for c in range(CPB):
                prefetch_x(0, c, eng=(nc.scalar if c % 2 else nc.sync))
            nc.scalar.dma_start(out=ones4q, in_=ones4q_d[:, :])
            nc.scalar.dma_start(out=ones4k, in_=ones4k_d[:, :])
            nc.sync.dma_start(out=wv_sb, in_=wv_d[:, :, :])
            nc.scalar.dma_start(out=ident, in_=ident_d[:, :])
            nc.scalar.dma_start(out=sel2, in_=sel2_d[:, :])
            nc.sync.dma_start(out=wo_sb, in_=wo_d[:, :])
            # pairs (not 4-wide): chunk 0's RoPE/squares chain starts ~2x
            # earlier, which gates the scales roundtrip -> first exp
            for pair in ((0, 1), (2, 3)):
                rr([p1_qk_gen(0, c, qtag=PRO_TAGS[c], ktag=PRO_TAGS[c],
                              tbufs=1, mm_step=4, act_sq=True)
                    for c in pair])
            rq = p1_rsqrt_gen(0)
            for _ in rq:
                pass
            for pair in ((0, 1), (2, 3)):
                rr([p1_v_gen(0, c, vtag=PRO_TAGS[c], tbufs=1, mm_step=4)
                    for c in pair])
            # apply LAST: its DVE muls wait on the scales DMA roundtrip;
            # emitting them earlier blocks the strict DVE FIFO (and its
            # mm512 bps allocs deadlock against the V transposes).
            for _ in p1_apply_gen(0):
                pass

            # ---- steady state ----
            tail = None
            for b in range(B):
                fillers = []
                if b + 1 < B:
                    fillers.append(p1_chain(b + 1))
                for qc in range(NQC):
                    if b + 1 < B and qc < CPB:
                        prefetch_x(b + 1, qc)
                    tail = attn_qc(b, qc, fillers, tail)
                # drain leftover filler quanta at the batch boundary
                for g in fillers:
                    for gen in g:
                        for _ in gen:
                            pass
                fillers.clear()
            # final qc's evacuation + norm + Wo
            if tail is not None:
                for gen in tail():
                    for _ in gen:
                        pass

    nc.compile()
    return nc


def make_core_inputs(x, cos, sin, Wq, Wk, Wv, Wo, B, T):
    """Host-side sharding. Returns list of 8 input dicts."""
    TT = B * T
    NCIN = C // 128
    # [128 ci, NCIN co, TT] so each x-chunk DMA reads contiguous rows
    xT = np.ascontiguousarray(
        np.asarray(x, np.float32).reshape(TT, C).T.astype(BF16NP)
        .reshape(NCIN, 128, TT).transpose(1, 0, 2))
    cosT = np.asarray(cos, np.float32).reshape(T, D).T      # [64, T]
    sinT = np.asarray(sin, np.float32).reshape(T, D).T
    sin_signed = np.concatenate([-sinT[0:32], sinT[32:64]], axis=0)
    # pre-permuted: sin2p[p] = sin_signed[partner(p)] (32-block swap)
    sin_perm = np.concatenate([sin_signed[32:64], sin_signed[0:32]], axis=0)
    cos2 = np.ascontiguousarray(np.concatenate([cosT, cosT], axis=0))
    sin2p = np.ascontiguousarray(np.concatenate([sin_perm, sin_perm],
                                                axis=0))
    ones4q = np.zeros((128, 4), BF16NP)
    ones4q[0:64, 0] = 1.0
    ones4q[64:128, 1] = 1.0
    ones4k = np.zeros((128, 4), BF16NP)
    ones4k[0:64, 2] = 1.0
    ones4k[64:128, 3] = 1.0
    sel2 = np.zeros((2, 128), BF16NP)
    sel2[0, 0:64] = 1.0
    sel2[1, 64:128] = 1.0

    def wprep(W, rows):
        # [128 ci, NCIN co, M2] so the weight DMA is contiguous
        wT = np.asarray(W, np.float32)[rows].T.astype(BF16NP)   # [C, M2]
        return np.ascontiguousarray(
            wT.reshape(NCIN, 128, M2).transpose(1, 0, 2))

    in_maps = []
    for core in range(N_CORES):
        rows = slice(core * M2, (core + 1) * M2)
        in_maps.append({
            "xT": xT,
            "wq": wprep(Wq, rows),
            "wk": wprep(Wk, rows),
            "wv": wprep(Wv, rows),
            "wo": np.ascontiguousarray(
                np.asarray(Wo, np.float32)[:, rows].T.astype(BF16NP)),
            "cos2": cos2,
            "sin2p": sin2p,
            "ident": np.eye(128, dtype=BF16NP),
            "ones4q": ones4q,
            "ones4k": ones4k,
            "sel2": sel2,
            "ones66": np.ones((128, T // 128, 66), BF16NP),
        })
    return in_maps


def kernel(x, cos, sin, Wq, Wk, Wv, Wo):
    from concourse.bass_utils import run_bass_kernel_spmd

    B, T = x.shape[0], x.shape[1]
    key = (B, T)
    if key not in _NC_CACHE:
        _NC_CACHE[key] = build_nc(B, T)
    nc = _NC_CACHE[key]
    in_maps = make_core_inputs(x, cos, sin, Wq, Wk, Wv, Wo, B, T)
    res = run_bass_kernel_spmd(nc, in_maps, core_ids=list(range(N_CORES)))
    out = np.zeros((B * T, C), np.float64)
    for r in res.results:
        out += r["out"].astype(np.float64)
    return out.astype(np.float32).reshape(B, T, C)
